# revision 17
# baseline (speedup 1.0000x reference)
"""MDCA loss kernel for Trainium2, 8 NeuronCores, data-parallel over batch.

reference:
    counts[c]   = histogram(target) ; avg_count = counts/B
    avg_conf    = mean(logits, axis=1)            # [E, C]
    loss[e]     = mean_c |avg_conf[e,c] - avg_count[c]|

Device computes ONLY per-core column sums of logits (the 16.4 MB/core
stream, which binds at the ~430 GB/s per-core HBM-read ceiling); the
target histogram (8 KB) and final abs/mean run on host.  A bf16
DGE-cast variant was measured: the HBM-read side still caps ~420 GB/s
and SWDGE's Q7 descriptor emission adds a long trickle tail, so pure
HWDGE f32 wins.

Per core (batch shard of 1024 rows, partition p holds rows 8p..8p+7):
  - two HWDGE rings (sync 8.70 MB / scalar 7.68 MB), 16 KB contiguous
    lines for the 4-row chunks.  Chunk sizes taper [4,4,4,2,2,1] /
    [4,4,4,2,1] rows so the end-game is short: the byte imbalance makes
    scalar's ring drain first, its closer matmuls run before sync's
    final single-row closer lands, leaving ~3 us of post-stream work.
  - DVE folds row pairs into [128,1000] f32r tiles (1.2 us each, 15
    total, always ahead of arrivals); single-row closers skip DVE.
  - ONE PSUM chain per 500-column half: f32r selector matmuls
    ([128,4] with ones in column e) fold rows+partitions:
    psum[e,c] = sum over shard of logits[e,:,c]
  - tail: bank0 closes one matmul before bank1; copy0 on DVE + store
    on sync overlap bank1's close, whose copy runs on ACT + scalar.
  - host: bincount(target); |sum_conf - counts|.mean / (B*C) -> loss[4]
"""

import os
import sys

for _p in ("/opt/trn_rl_repo", "/root/.axon_site/_ro/trn_rl_repo"):
    if os.path.isdir(_p) and _p not in sys.path:
        sys.path.insert(0, _p)

import numpy as np

import concourse.bass as bass
import concourse.bacc as bacc
import concourse.tile as tile
import concourse.mybir as mybir
from concourse.bass_utils import run_bass_kernel_spmd

E, B, C = 4, 8192, 1000
N_CORES = 8
BS = B // N_CORES          # 1024 batch rows per core
GP = 8                     # rows folded per partition (BS = 128 * GP)
CH = C // 2                # 500, C half per PSUM bank
F32 = mybir.dt.float32
F32R = mybir.dt.float32r
BF16 = mybir.dt.bfloat16

# (exit, row0, row1, col0, col1) in issue order per ring.  The scalar
# ring's HWDGE descriptor generation lags sync's by ~3 us when sync goes
# first, so scalar issues FIRST and carries the extra bytes.  Chunk rows
# taper [4,4,4,2,1,1,.5,.5] / [4,4,4,2,.5,.5] so the end-game is four
# half-row closers at one matmul each.
RING_S = [(0, 0, 4, 0, C), (1, 0, 4, 0, C), (2, 0, 4, 0, C),
          (3, 0, 2, 0, C), (3, 2, 4, 0, C), (3, 4, 6, 0, C),
          (3, 6, 7, 0, CH), (3, 6, 7, CH, C)]
RING_C = [(0, 4, 8, 0, C), (1, 4, 8, 0, C), (2, 4, 8, 0, C),
          (3, 7, 8, 0, C)]
# compute order ~ arrival order (sync starts ~3 us before scalar's ring
# drains its first packets; sync carries the surplus so it ends last,
# tapering 2-row chunks into two half-row closers)
ORDER = ["s0", "c0", "s1", "c1", "s2", "c2", "s3", "c3", "s4", "s5",
         "s6", "s7"]


def build_nc():
    nc = bacc.Bacc(
        "TRN2",
        target_bir_lowering=False,
        debug=False,
        enable_asserts=False,
        num_devices=N_CORES,
    )

    logits = nc.dram_tensor("logits", [E, BS, C], F32, kind="ExternalInput")
    part_out = nc.dram_tensor("part", [E, C], F32, kind="ExternalOutput")

    # per-exit view: partition p holds rows 8p..8p+7
    src = [logits[e].rearrange("(p i) c -> p i c", i=GP) for e in range(E)]

    with tile.TileContext(nc) as tc:
        with (
            tc.tile_pool(name="const", bufs=1) as const,
            tc.tile_pool(name="ld4", bufs=6) as ld4,
            tc.tile_pool(name="ld2", bufs=3) as ld2,
            tc.tile_pool(name="ldz", bufs=4) as ldz,
            tc.tile_pool(name="fold", bufs=4) as foldp,
            tc.tile_pool(name="work", bufs=2) as work,
            tc.tile_pool(name="psum", bufs=1, space=bass.MemorySpace.PSUM) as psum,
        ):
            # ---- phase 1: every load DMA first so both rings fill and
            # stream back-to-back
            def ld_dma(engine, key, e, r0, r1, c0, c1):
                rows, cols = r1 - r0, c1 - c0
                if rows == 1:           # closer: f32r bitcast, no fold
                    pool, dt = ldz, F32R
                else:
                    pool, dt = (ld4 if rows == 4 else ld2), F32
                t = pool.tile([128, rows * cols], dt,
                              tag=f"ld{rows}_{cols}", name=f"ld_{key}")
                in_ = src[e][:, r0:r1, c0:c1]
                if dt is F32R:
                    in_ = in_.bitcast(F32R)
                engine.dma_start(
                    out=t.rearrange("p (i c) -> p i c", i=rows), in_=in_
                )
                return t

            tiles = {}
            for k, spec in enumerate(RING_C):
                tiles[f"c{k}"] = (ld_dma(nc.scalar, f"c{k}", *spec), spec)
            for k, spec in enumerate(RING_S):
                tiles[f"s{k}"] = (ld_dma(nc.sync, f"s{k}", *spec), spec)

            # ---- phase 2: selector weights (bf16 for folded tiles,
            # f32r for closers)
            sels_b = const.tile([128, 4 * E], BF16, tag="sels_b")
            nc.vector.memset(sels_b[:], 0.0)
            for e in range(E):
                nc.vector.memset(sels_b[:, 4 * e + e : 4 * e + e + 1], 1.0)
            sels_f = const.tile([128, 4 * E], F32, tag="sels_f")
            nc.vector.tensor_copy(sels_f[:], sels_b[:])
            sels_r_t = const.tile([128, 4 * E], F32R, tag="sels_r")
            nc.vector.tensor_copy(sels_r_t[:], sels_f[:])
            sels_r = sels_r_t[:]

            pbank = [
                psum.tile([E, CH], F32, tag=f"pc{h}", name=f"pc{h}")
                for h in range(2)
            ]

            # ---- phase 3: folds + matmuls in expected arrival order.
            # Precount per-bank matmuls so the last one sets stop=True.
            tot = [0, 0]
            for _, r0, r1, c0, c1 in RING_S + RING_C:
                rows = r1 - r0
                if c1 - c0 == CH:
                    tot[c0 // CH] += 1
                else:
                    tot[0] += rows // 2 + rows % 2
                    tot[1] += rows // 2 + rows % 2
            n_mm = [0, 0]

            def mm1(h, w, data):
                nc.tensor.matmul(
                    pbank[h][:], w, data,
                    start=(n_mm[h] == 0),
                    stop=(n_mm[h] == tot[h] - 1),
                )
                n_mm[h] += 1

            for key in ORDER:
                t, (e, r0, r1, c0, c1) = tiles[key]
                rows = r1 - r0
                wb = sels_b[:, 4 * e : 4 * e + 4]
                wr = sels_r[:, 4 * e : 4 * e + 4]
                if rows == 1 and c1 - c0 == CH:     # half-row closer
                    mm1(c0 // CH, wr, t[:, 0:CH])
                elif rows == 1:                     # full-row closer
                    for h in range(2):
                        mm1(h, wr, t[:, h * CH : (h + 1) * CH])
                else:
                    # all folds on DVE (gpsimd tensor_add measured 2.3-3.3
                    # us per [128,1000] fold vs DVE's 1.2 — never worth it)
                    fold_eng = nc.vector
                    for g in range(rows // 2):
                        f = foldp.tile([128, C], BF16, tag="fold",
                                       name=f"f_{key}g{g}")
                        fold_eng.tensor_add(
                            f[:], t[:, 2 * g * C : (2 * g + 1) * C],
                            t[:, (2 * g + 1) * C : (2 * g + 2) * C],
                        )
                        for h in range(2):
                            mm1(h, wb, f[:, h * CH : (h + 1) * CH])

            # ---- phase 4: PSUM->SBUF on two engines + parallel stores
            sb0 = work.tile([E, CH], F32, tag="sb0")
            nc.vector.tensor_copy(sb0[:], pbank[0][:])
            nc.sync.dma_start(out=part_out[:, 0:CH], in_=sb0[:])
            sb1 = work.tile([E, CH], F32, tag="sb1")
            nc.scalar.copy(sb1[:], pbank[1][:])
            nc.scalar.dma_start(out=part_out[:, CH:C], in_=sb1[:])

    nc.compile()
    return nc


_NC_CACHE = {}


def _get_nc():
    if "nc" not in _NC_CACHE:
        _NC_CACHE["nc"] = build_nc()
    return _NC_CACHE["nc"]


def make_in_maps(logits: np.ndarray, target: np.ndarray):
    logits = np.ascontiguousarray(logits, dtype=np.float32)
    in_maps = []
    for c in range(N_CORES):
        lg = logits[:, c * BS : (c + 1) * BS, :]
        in_maps.append({"logits": np.ascontiguousarray(lg)})
    return in_maps


def kernel(logits: np.ndarray, target: np.ndarray) -> np.ndarray:
    nc = _get_nc()
    in_maps = make_in_maps(logits, target)
    res = run_bass_kernel_spmd(nc, in_maps, core_ids=list(range(N_CORES)))
    parts = sum(np.asarray(r["part"], dtype=np.float64) for r in res.results)
    counts = np.bincount(
        np.asarray(target).astype(np.int64), minlength=C
    ).astype(np.float64)
    return (np.abs(parts - counts[None, :]).sum(axis=1) / (B * C)).astype(
        np.float32
    )


# revision 18
# speedup vs baseline: 1.1761x; 1.1761x over previous
"""MDCA loss kernel for Trainium2, 8 NeuronCores, data-parallel over batch.

reference:
    counts[c]   = histogram(target) ; avg_count = counts/B
    avg_conf    = mean(logits, axis=1)            # [E, C]
    loss[e]     = mean_c |avg_conf[e,c] - avg_count[c]|

Device computes ONLY per-core column sums of logits (the 16.4 MB/core
stream, which binds at the ~430 GB/s per-core HBM-read ceiling); the
target histogram (8 KB) and final abs/mean run on host.  A bf16
DGE-cast variant was measured: the HBM-read side still caps ~420 GB/s
and SWDGE's Q7 descriptor emission adds a long trickle tail, so pure
HWDGE f32 wins.

Per core (batch shard of 1024 rows, partition p holds rows 8p..8p+7):
  - two HWDGE rings (sync 8.70 MB / scalar 7.68 MB), 16 KB contiguous
    lines for the 4-row chunks.  Chunk sizes taper [4,4,4,2,2,1] /
    [4,4,4,2,1] rows so the end-game is short: the byte imbalance makes
    scalar's ring drain first, its closer matmuls run before sync's
    final single-row closer lands, leaving ~3 us of post-stream work.
  - DVE folds row pairs into [128,1000] f32r tiles (1.2 us each, 15
    total, always ahead of arrivals); single-row closers skip DVE.
  - ONE PSUM chain per 500-column half: f32r selector matmuls
    ([128,4] with ones in column e) fold rows+partitions:
    psum[e,c] = sum over shard of logits[e,:,c]
  - tail: bank0 closes one matmul before bank1; copy0 on DVE + store
    on sync overlap bank1's close, whose copy runs on ACT + scalar.
  - host: bincount(target); |sum_conf - counts|.mean / (B*C) -> loss[4]
"""

import os
import sys

for _p in ("/opt/trn_rl_repo", "/root/.axon_site/_ro/trn_rl_repo"):
    if os.path.isdir(_p) and _p not in sys.path:
        sys.path.insert(0, _p)

import numpy as np

import concourse.bass as bass
import concourse.bacc as bacc
import concourse.tile as tile
import concourse.mybir as mybir
from concourse.bass_utils import run_bass_kernel_spmd

E, B, C = 4, 8192, 1000
N_CORES = 8
BS = B // N_CORES          # 1024 batch rows per core
GP = 8                     # rows folded per partition (BS = 128 * GP)
CH = C // 2                # 500, C half per PSUM bank
F32 = mybir.dt.float32
F32R = mybir.dt.float32r
BF16 = mybir.dt.bfloat16

# (exit, row0, row1, col0, col1) in issue order per ring.  The scalar
# ring's HWDGE descriptor generation lags sync's by ~3 us when sync goes
# first, so scalar issues FIRST and carries the extra bytes.  Chunk rows
# taper [4,4,4,2,1,1,.5,.5] / [4,4,4,2,.5,.5] so the end-game is four
# half-row closers at one matmul each.
RING_S = [(0, 0, 4, 0, C), (1, 0, 4, 0, C), (2, 0, 4, 0, C),
          (3, 0, 2, 0, C), (3, 2, 4, 0, C), (3, 4, 6, 0, C),
          (3, 6, 7, 0, CH), (3, 6, 7, CH, C)]
RING_C = [(0, 4, 8, 0, C), (1, 4, 8, 0, C), (2, 4, 8, 0, C),
          (3, 7, 8, 0, C)]
# compute order ~ arrival order (sync starts ~3 us before scalar's ring
# drains its first packets; sync carries the surplus so it ends last,
# tapering 2-row chunks into two half-row closers)
ORDER = ["s0", "c0", "s1", "c1", "s2", "c2", "s3", "c3", "s4", "s5",
         "s6", "s7"]


def build_nc():
    nc = bacc.Bacc(
        "TRN2",
        target_bir_lowering=False,
        debug=False,
        enable_asserts=False,
        num_devices=N_CORES,
    )

    logits = nc.dram_tensor("logits", [E, BS, C], F32, kind="ExternalInput")
    part_out = nc.dram_tensor("part", [E, C], F32, kind="ExternalOutput")

    # per-exit view: partition p holds rows 8p..8p+7
    src = [logits[e].rearrange("(p i) c -> p i c", i=GP) for e in range(E)]

    with tile.TileContext(nc) as tc:
        with (
            tc.tile_pool(name="const", bufs=1) as const,
            tc.tile_pool(name="ld4", bufs=6) as ld4,
            tc.tile_pool(name="ld2", bufs=3) as ld2,
            tc.tile_pool(name="ldz", bufs=4) as ldz,
            tc.tile_pool(name="fold", bufs=4) as foldp,
            tc.tile_pool(name="work", bufs=2) as work,
            tc.tile_pool(name="psum", bufs=1, space=bass.MemorySpace.PSUM) as psum,
        ):
            # ---- phase 1: every load DMA first so both rings fill and
            # stream back-to-back
            def ld_dma(engine, key, e, r0, r1, c0, c1):
                rows, cols = r1 - r0, c1 - c0
                if rows == 1:           # closer: f32r bitcast, no fold
                    pool, dt = ldz, F32R
                else:
                    pool, dt = (ld4 if rows == 4 else ld2), F32
                t = pool.tile([128, rows * cols], dt,
                              tag=f"ld{rows}_{cols}", name=f"ld_{key}")
                in_ = src[e][:, r0:r1, c0:c1]
                if dt is F32R:
                    in_ = in_.bitcast(F32R)
                engine.dma_start(
                    out=t.rearrange("p (i c) -> p i c", i=rows), in_=in_
                )
                return t

            # Issue order interleaves the rings: DMAHW completion-sem
            # lanes (8) are assigned round-robin in emission order, and a
            # recycled lane's issue waits for its previous transfer to
            # COMPLETE.  Interleaving makes every recycled lane wait on an
            # early-completing transfer, so the tail transfers' descriptors
            # always ring long before the SDMA engines reach them.
            # Per-engine ring order (= arrival order) is still s0<s1<...
            ISSUE = ["c0", "s0", "c1", "s1", "c2", "s2", "c3", "s3",
                     "s4", "s5", "s6", "s7"]
            spec_of = {f"c{k}": ("c", spec) for k, spec in enumerate(RING_C)}
            spec_of.update(
                {f"s{k}": ("s", spec) for k, spec in enumerate(RING_S)}
            )
            tiles = {}
            for key in ISSUE:
                ring, spec = spec_of[key]
                engine = nc.sync if ring == "s" else nc.scalar
                tiles[key] = (ld_dma(engine, key, *spec), spec)

            # ---- phase 2: selector weights (bf16 for folded tiles,
            # f32r for closers)
            sels_b = const.tile([128, 4 * E], BF16, tag="sels_b")
            nc.vector.memset(sels_b[:], 0.0)
            for e in range(E):
                nc.vector.memset(sels_b[:, 4 * e + e : 4 * e + e + 1], 1.0)
            sels_f = const.tile([128, 4 * E], F32, tag="sels_f")
            nc.vector.tensor_copy(sels_f[:], sels_b[:])
            sels_r_t = const.tile([128, 4 * E], F32R, tag="sels_r")
            nc.vector.tensor_copy(sels_r_t[:], sels_f[:])
            sels_r = sels_r_t[:]

            pbank = [
                psum.tile([E, CH], F32, tag=f"pc{h}", name=f"pc{h}")
                for h in range(2)
            ]

            # ---- phase 3: folds + matmuls in expected arrival order.
            # Precount per-bank matmuls so the last one sets stop=True.
            tot = [0, 0]
            for _, r0, r1, c0, c1 in RING_S + RING_C:
                rows = r1 - r0
                if c1 - c0 == CH:
                    tot[c0 // CH] += 1
                else:
                    tot[0] += rows // 2 + rows % 2
                    tot[1] += rows // 2 + rows % 2
            n_mm = [0, 0]

            def mm1(h, w, data):
                nc.tensor.matmul(
                    pbank[h][:], w, data,
                    start=(n_mm[h] == 0),
                    stop=(n_mm[h] == tot[h] - 1),
                )
                n_mm[h] += 1

            for key in ORDER:
                t, (e, r0, r1, c0, c1) = tiles[key]
                rows = r1 - r0
                wb = sels_b[:, 4 * e : 4 * e + 4]
                wr = sels_r[:, 4 * e : 4 * e + 4]
                if rows == 1 and c1 - c0 == CH:     # half-row closer
                    mm1(c0 // CH, wr, t[:, 0:CH])
                elif rows == 1:                     # full-row closer
                    for h in range(2):
                        mm1(h, wr, t[:, h * CH : (h + 1) * CH])
                else:
                    # all folds on DVE (gpsimd tensor_add measured 2.3-3.3
                    # us per [128,1000] fold vs DVE's 1.2 — never worth it)
                    fold_eng = nc.vector
                    for g in range(rows // 2):
                        f = foldp.tile([128, C], BF16, tag="fold",
                                       name=f"f_{key}g{g}")
                        fold_eng.tensor_add(
                            f[:], t[:, 2 * g * C : (2 * g + 1) * C],
                            t[:, (2 * g + 1) * C : (2 * g + 2) * C],
                        )
                        for h in range(2):
                            mm1(h, wb, f[:, h * CH : (h + 1) * CH])

            # ---- phase 4: PSUM->SBUF on two engines + parallel stores
            sb0 = work.tile([E, CH], F32, tag="sb0")
            nc.vector.tensor_copy(sb0[:], pbank[0][:])
            nc.sync.dma_start(out=part_out[:, 0:CH], in_=sb0[:])
            sb1 = work.tile([E, CH], F32, tag="sb1")
            nc.scalar.copy(sb1[:], pbank[1][:])
            nc.scalar.dma_start(out=part_out[:, CH:C], in_=sb1[:])

    nc.compile()
    return nc


_NC_CACHE = {}


def _get_nc():
    if "nc" not in _NC_CACHE:
        _NC_CACHE["nc"] = build_nc()
    return _NC_CACHE["nc"]


def make_in_maps(logits: np.ndarray, target: np.ndarray):
    logits = np.ascontiguousarray(logits, dtype=np.float32)
    in_maps = []
    for c in range(N_CORES):
        lg = logits[:, c * BS : (c + 1) * BS, :]
        in_maps.append({"logits": np.ascontiguousarray(lg)})
    return in_maps


def kernel(logits: np.ndarray, target: np.ndarray) -> np.ndarray:
    nc = _get_nc()
    in_maps = make_in_maps(logits, target)
    res = run_bass_kernel_spmd(nc, in_maps, core_ids=list(range(N_CORES)))
    parts = sum(np.asarray(r["part"], dtype=np.float64) for r in res.results)
    counts = np.bincount(
        np.asarray(target).astype(np.int64), minlength=C
    ).astype(np.float64)
    return (np.abs(parts - counts[None, :]).sum(axis=1) / (B * C)).astype(
        np.float32
    )


# revision 21
# speedup vs baseline: 1.2144x; 1.0326x over previous
"""MDCA loss kernel for Trainium2, 8 NeuronCores, data-parallel over batch.

reference:
    counts[c]   = histogram(target) ; avg_count = counts/B
    avg_conf    = mean(logits, axis=1)            # [E, C]
    loss[e]     = mean_c |avg_conf[e,c] - avg_count[c]|

Device computes ONLY per-core column sums of logits (the 16.4 MB/core
stream, which binds at the ~430 GB/s per-core HBM-read ceiling); the
target histogram (8 KB) and final abs/mean run on host.  A bf16
DGE-cast variant was measured: the HBM-read side still caps ~420 GB/s
and SWDGE's Q7 descriptor emission adds a long trickle tail, so pure
HWDGE f32 wins.

Per core (batch shard of 1024 rows, partition p holds rows 8p..8p+7):
  - two HWDGE rings stream 16 KB contiguous lines; chunk sizes taper
    so the end-game is single-row f32r closers and four half-row
    closers at one matmul each.
  - DVE folds row pairs into [128,1000] bf16 tiles (1.2 us each,
    paced by arrivals); closers skip DVE.
  - ONE PSUM chain per 500-column half: selector matmuls ([128,4]
    with ones in column e) fold rows+partitions:
    psum[e,c] = sum over shard of logits[e,:,c]
  - tail: bank0 closes before bank1; copy0 on DVE + store on sync
    overlap bank1's close, whose copy runs on ACT + scalar.
  - host: bincount(target); |sum_conf - counts|.mean / (B*C) -> loss[4]
"""

import os
import sys

for _p in ("/opt/trn_rl_repo", "/root/.axon_site/_ro/trn_rl_repo"):
    if os.path.isdir(_p) and _p not in sys.path:
        sys.path.insert(0, _p)

import numpy as np

import concourse.bass as bass
import concourse.bacc as bacc
import concourse.tile as tile
import concourse.mybir as mybir
from concourse.bass_utils import run_bass_kernel_spmd

E, B, C = 4, 8192, 1000
N_CORES = 8
BS = B // N_CORES          # 1024 batch rows per core
GP = 8                     # rows folded per partition (BS = 128 * GP)
CH = C // 2                # 500, C half per PSUM bank
F32 = mybir.dt.float32
F32R = mybir.dt.float32r
BF16 = mybir.dt.bfloat16

# (exit, row0, row1, col0, col1) in issue order per ring.
RING_C = [(0, 4, 8, 0, C), (1, 4, 8, 0, C), (2, 4, 8, 0, C),
          (3, 4, 6, 0, C), (3, 6, 7, 0, C), (3, 7, 8, 0, C),
          (3, 3, 4, 0, CH), (3, 3, 4, CH, C)]
RING_S = [(0, 0, 4, 0, C), (1, 0, 4, 0, C), (2, 0, 4, 0, C),
          (3, 0, 2, 0, C), (3, 2, 3, 0, CH), (3, 2, 3, CH, C)]
# compute order ~ arrival order; half-closers land last
ORDER = ["c0", "s0", "c1", "s1", "c2", "s2", "c3", "s3", "c4", "c5",
         "s4", "c6", "s5", "c7"]


def build_nc():
    nc = bacc.Bacc(
        "TRN2",
        target_bir_lowering=False,
        debug=False,
        enable_asserts=False,
        num_devices=N_CORES,
    )

    logits = nc.dram_tensor("logits", [E, BS, C], F32, kind="ExternalInput")
    part_out = nc.dram_tensor("part", [E, C], F32, kind="ExternalOutput")

    # per-exit view: partition p holds rows 8p..8p+7
    src = [logits[e].rearrange("(p i) c -> p i c", i=GP) for e in range(E)]

    with tile.TileContext(nc) as tc:
        with (
            tc.tile_pool(name="const", bufs=1) as const,
            tc.tile_pool(name="ld4", bufs=6) as ld4,
            tc.tile_pool(name="ld2", bufs=3) as ld2,
            tc.tile_pool(name="ldz", bufs=4) as ldz,
            tc.tile_pool(name="fold", bufs=4) as foldp,
            tc.tile_pool(name="work", bufs=2) as work,
            tc.tile_pool(name="psum", bufs=1, space=bass.MemorySpace.PSUM) as psum,
        ):
            # ---- phase 1: every load DMA first so both rings fill and
            # stream back-to-back
            def ld_dma(engine, key, e, r0, r1, c0, c1):
                rows, cols = r1 - r0, c1 - c0
                if rows == 1:           # closer: f32r bitcast, no fold
                    pool, dt = ldz, F32R
                else:
                    pool, dt = (ld4 if rows == 4 else ld2), F32
                t = pool.tile([128, rows * cols], dt,
                              tag=f"ld{rows}_{cols}", name=f"ld_{key}")
                in_ = src[e][:, r0:r1, c0:c1]
                if dt is F32R:
                    in_ = in_.bitcast(F32R)
                engine.dma_start(
                    out=t.rearrange("p (i c) -> p i c", i=rows), in_=in_
                )
                return t

            tiles = {}
            for k, spec in enumerate(RING_C):
                tiles[f"c{k}"] = (ld_dma(nc.scalar, f"c{k}", *spec), spec)
            for k, spec in enumerate(RING_S):
                tiles[f"s{k}"] = (ld_dma(nc.sync, f"s{k}", *spec), spec)

            # ---- phase 2: selector weights (bf16 for folded tiles,
            # f32r for closers)
            sels_b = const.tile([128, 4 * E], BF16, tag="sels_b")
            nc.vector.memset(sels_b[:], 0.0)
            for e in range(E):
                nc.vector.memset(sels_b[:, 4 * e + e : 4 * e + e + 1], 1.0)
            sels_f = const.tile([128, 4 * E], F32, tag="sels_f")
            nc.vector.tensor_copy(sels_f[:], sels_b[:])
            # real F32R tile via copy — walrus's checkMatmultFP32r
            # rejects a bitcast AP as matmul weights
            sels_r_t = const.tile([128, 4 * E], F32R, tag="sels_r")
            nc.vector.tensor_copy(sels_r_t[:], sels_f[:])
            sels_r = sels_r_t[:]

            pbank = [
                psum.tile([E, CH], F32, tag=f"pc{h}", name=f"pc{h}")
                for h in range(2)
            ]

            # ---- phase 3: folds + matmuls in expected arrival order.
            # Precount per-bank matmuls so the last one sets stop=True.
            tot = [0, 0]
            for _, r0, r1, c0, c1 in RING_S + RING_C:
                rows = r1 - r0
                if c1 - c0 == CH:
                    tot[c0 // CH] += 1
                else:
                    tot[0] += rows // 2 + rows % 2
                    tot[1] += rows // 2 + rows % 2
            n_mm = [0, 0]

            def mm1(h, w, data):
                nc.tensor.matmul(
                    pbank[h][:], w, data,
                    start=(n_mm[h] == 0),
                    stop=(n_mm[h] == tot[h] - 1),
                )
                n_mm[h] += 1

            for key in ORDER:
                t, (e, r0, r1, c0, c1) = tiles[key]
                rows = r1 - r0
                wb = sels_b[:, 4 * e : 4 * e + 4]
                wr = sels_r[:, 4 * e : 4 * e + 4]
                if rows == 1 and c1 - c0 == CH:     # half-row closer
                    mm1(c0 // CH, wr, t[:, 0:CH])
                elif rows == 1:                     # full-row closer
                    for h in range(2):
                        mm1(h, wr, t[:, h * CH : (h + 1) * CH])
                else:
                    for g in range(rows // 2):
                        f = foldp.tile([128, C], BF16, tag="fold",
                                       name=f"f_{key}g{g}")
                        nc.vector.tensor_add(
                            f[:], t[:, 2 * g * C : (2 * g + 1) * C],
                            t[:, (2 * g + 1) * C : (2 * g + 2) * C],
                        )
                        for h in range(2):
                            mm1(h, wb, f[:, h * CH : (h + 1) * CH])

            # ---- phase 4: PSUM->SBUF on two engines + parallel stores
            sb0 = work.tile([E, CH], F32, tag="sb0")
            nc.vector.tensor_copy(sb0[:], pbank[0][:])
            nc.sync.dma_start(out=part_out[:, 0:CH], in_=sb0[:])
            sb1 = work.tile([E, CH], F32, tag="sb1")
            nc.scalar.copy(sb1[:], pbank[1][:])
            nc.scalar.dma_start(out=part_out[:, CH:C], in_=sb1[:])

    nc.compile()
    return nc


_NC_CACHE = {}


def _get_nc():
    if "nc" not in _NC_CACHE:
        _NC_CACHE["nc"] = build_nc()
    return _NC_CACHE["nc"]


def make_in_maps(logits: np.ndarray, target: np.ndarray):
    logits = np.ascontiguousarray(logits, dtype=np.float32)
    in_maps = []
    for c in range(N_CORES):
        lg = logits[:, c * BS : (c + 1) * BS, :]
        in_maps.append({"logits": np.ascontiguousarray(lg)})
    return in_maps


def kernel(logits: np.ndarray, target: np.ndarray) -> np.ndarray:
    nc = _get_nc()
    in_maps = make_in_maps(logits, target)
    res = run_bass_kernel_spmd(nc, in_maps, core_ids=list(range(N_CORES)))
    parts = sum(np.asarray(r["part"], dtype=np.float64) for r in res.results)
    counts = np.bincount(
        np.asarray(target).astype(np.int64), minlength=C
    ).astype(np.float64)
    return (np.abs(parts - counts[None, :]).sum(axis=1) / (B * C)).astype(
        np.float32
    )


# revision 26
# speedup vs baseline: 1.2164x; 1.0016x over previous
"""MDCA loss kernel for Trainium2, 8 NeuronCores, data-parallel over batch.

reference:
    counts[c]   = histogram(target) ; avg_count = counts/B
    avg_conf    = mean(logits, axis=1)            # [E, C]
    loss[e]     = mean_c |avg_conf[e,c] - avg_count[c]|

Device computes ONLY per-core column sums of logits (the 16.4 MB/core
stream, which binds at the ~430 GB/s per-core HBM-read ceiling); the
target histogram (8 KB) and final abs/mean run on host.  A bf16
DGE-cast variant was measured: the HBM-read side still caps ~420 GB/s
and SWDGE's Q7 descriptor emission adds a long trickle tail, so pure
HWDGE f32 wins.

Per core (batch shard of 1024 rows, partition p holds rows 8p..8p+7):
  - two HWDGE rings stream 16 KB contiguous lines; chunk sizes taper
    so the end-game is single-row f32r closers and four half-row
    closers at one matmul each.
  - DVE folds row pairs into [128,1000] bf16 tiles (1.2 us each,
    paced by arrivals); closers skip DVE.
  - ONE PSUM chain per 500-column half: selector matmuls ([128,4]
    with ones in column e) fold rows+partitions:
    psum[e,c] = sum over shard of logits[e,:,c]
  - tail: bank0 closes before bank1; copy0 on DVE + store on sync
    overlap bank1's close, whose copy runs on ACT + scalar.
  - host: bincount(target); |sum_conf - counts|.mean / (B*C) -> loss[4]
"""

import os
import sys

for _p in ("/opt/trn_rl_repo", "/root/.axon_site/_ro/trn_rl_repo"):
    if os.path.isdir(_p) and _p not in sys.path:
        sys.path.insert(0, _p)

import numpy as np

import concourse.bass as bass
import concourse.bacc as bacc
import concourse.tile as tile
import concourse.mybir as mybir
from concourse.bass_utils import run_bass_kernel_spmd

E, B, C = 4, 8192, 1000
N_CORES = 8
BS = B // N_CORES          # 1024 batch rows per core
GP = 8                     # rows folded per partition (BS = 128 * GP)
CH = C // 2                # 500, C half per PSUM bank
F32 = mybir.dt.float32
F32R = mybir.dt.float32r
BF16 = mybir.dt.bfloat16

# (exit, row0, row1, col0, col1) in issue order per ring.
RING_C = [(0, 4, 8, 0, C), (1, 4, 8, 0, C), (2, 4, 8, 0, C),
          (3, 4, 6, 0, C), (3, 6, 7, 0, C), (3, 7, 8, 0, C),
          (3, 3, 4, 0, CH), (3, 3, 4, CH, C)]
RING_S = [(0, 0, 4, 0, C), (1, 0, 4, 0, C), (2, 0, 4, 0, C),
          (3, 0, 2, 0, C), (3, 2, 3, 0, CH), (3, 2, 3, CH, C)]
# compute order ~ arrival order; half-closers land last
ORDER = ["c0", "s0", "c1", "s1", "c2", "s2", "c3", "s3", "c4", "c5",
         "s4", "c6", "s5", "c7"]


def build_nc():
    nc = bacc.Bacc(
        "TRN2",
        target_bir_lowering=False,
        debug=False,
        enable_asserts=False,
        num_devices=N_CORES,
    )

    logits = nc.dram_tensor("logits", [E, BS, C], F32, kind="ExternalInput")
    part_out = nc.dram_tensor("part", [E, C], F32, kind="ExternalOutput")

    # per-exit view: partition p holds rows 8p..8p+7
    src = [logits[e].rearrange("(p i) c -> p i c", i=GP) for e in range(E)]

    with tile.TileContext(nc) as tc:
        with (
            tc.tile_pool(name="const", bufs=1) as const,
            tc.tile_pool(name="ld4", bufs=6) as ld4,
            tc.tile_pool(name="ld2", bufs=3) as ld2,
            tc.tile_pool(name="ldz", bufs=4) as ldz,
            tc.tile_pool(name="fold", bufs=4) as foldp,
            tc.tile_pool(name="work", bufs=2) as work,
            tc.tile_pool(name="psum", bufs=1, space=bass.MemorySpace.PSUM) as psum,
        ):
            # ---- phase 1: every load DMA first so both rings fill and
            # stream back-to-back
            def ld_dma(engine, key, e, r0, r1, c0, c1):
                rows, cols = r1 - r0, c1 - c0
                if rows == 1:           # closer: f32r bitcast, no fold
                    pool, dt = ldz, F32R
                else:
                    pool, dt = (ld4 if rows == 4 else ld2), F32
                t = pool.tile([128, rows * cols], dt,
                              tag=f"ld{rows}_{cols}", name=f"ld_{key}")
                in_ = src[e][:, r0:r1, c0:c1]
                if dt is F32R:
                    in_ = in_.bitcast(F32R)
                engine.dma_start(
                    out=t.rearrange("p (i c) -> p i c", i=rows), in_=in_
                )
                return t

            tiles = {}
            for k, spec in enumerate(RING_C):
                tiles[f"c{k}"] = (ld_dma(nc.scalar, f"c{k}", *spec), spec)
            for k, spec in enumerate(RING_S):
                tiles[f"s{k}"] = (ld_dma(nc.sync, f"s{k}", *spec), spec)

            # ---- phase 2: selector weights (bf16 for folded tiles,
            # f32r for closers)
            sels_b = const.tile([128, 4 * E], BF16, tag="sels_b")
            nc.vector.memset(sels_b[:], 0.0)
            for e in range(E):
                nc.vector.memset(sels_b[:, 4 * e + e : 4 * e + e + 1], 1.0)
            sels_f = const.tile([128, 4 * E], F32, tag="sels_f")
            nc.vector.tensor_copy(sels_f[:], sels_b[:])
            # real F32R tile via copy — walrus's checkMatmultFP32r
            # rejects a bitcast AP as matmul weights
            sels_r_t = const.tile([128, 4 * E], F32R, tag="sels_r")
            nc.vector.tensor_copy(sels_r_t[:], sels_f[:])
            sels_r = sels_r_t[:]

            pbank = [
                psum.tile([E, CH], F32, tag=f"pc{h}", name=f"pc{h}")
                for h in range(2)
            ]

            # ---- phase 3: folds + matmuls in expected arrival order.
            # Precount per-bank matmuls so the last one sets stop=True.
            tot = [0, 0]
            for _, r0, r1, c0, c1 in RING_S + RING_C:
                rows = r1 - r0
                if c1 - c0 == CH:
                    tot[c0 // CH] += 1
                else:
                    tot[0] += rows // 2 + rows % 2
                    tot[1] += rows // 2 + rows % 2
            n_mm = [0, 0]

            def mm1(h, w, data):
                nc.tensor.matmul(
                    pbank[h][:], w, data,
                    start=(n_mm[h] == 0),
                    stop=(n_mm[h] == tot[h] - 1),
                )
                n_mm[h] += 1

            for key in ORDER:
                t, (e, r0, r1, c0, c1) = tiles[key]
                rows = r1 - r0
                wb = sels_b[:, 4 * e : 4 * e + 4]
                wr = sels_r[:, 4 * e : 4 * e + 4]
                if rows == 1 and c1 - c0 == CH:     # half-row closer
                    mm1(c0 // CH, wr, t[:, 0:CH])
                elif rows == 1:                     # full-row closer
                    for h in range(2):
                        mm1(h, wr, t[:, h * CH : (h + 1) * CH])
                else:
                    for g in range(rows // 2):
                        f = foldp.tile([128, C], BF16, tag="fold",
                                       name=f"f_{key}g{g}")
                        nc.vector.tensor_add(
                            f[:], t[:, 2 * g * C : (2 * g + 1) * C],
                            t[:, (2 * g + 1) * C : (2 * g + 2) * C],
                        )
                        for h in range(2):
                            mm1(h, wb, f[:, h * CH : (h + 1) * CH])

            # ---- phase 4: PSUM->SBUF on two engines + parallel stores
            sb0 = work.tile([E, CH], F32, tag="sb0")
            nc.vector.tensor_copy(sb0[:], pbank[0][:])
            nc.sync.dma_start(out=part_out[:, 0:CH], in_=sb0[:])
            sb1 = work.tile([E, CH], F32, tag="sb1")
            nc.scalar.copy(sb1[:], pbank[1][:])
            nc.scalar.dma_start(out=part_out[:, CH:C], in_=sb1[:])

    nc.compile()
    return nc


_NC_CACHE = {}


def _get_nc():
    if "nc" not in _NC_CACHE:
        _NC_CACHE["nc"] = build_nc()
    return _NC_CACHE["nc"]


def make_in_maps(logits: np.ndarray, target: np.ndarray):
    logits = np.ascontiguousarray(logits, dtype=np.float32)
    in_maps = []
    for c in range(N_CORES):
        lg = logits[:, c * BS : (c + 1) * BS, :]
        in_maps.append({"logits": np.ascontiguousarray(lg)})
    return in_maps


def kernel(logits: np.ndarray, target: np.ndarray) -> np.ndarray:
    nc = _get_nc()
    in_maps = make_in_maps(logits, target)
    res = run_bass_kernel_spmd(nc, in_maps, core_ids=list(range(N_CORES)))
    parts = sum(np.asarray(r["part"], dtype=np.float64) for r in res.results)
    counts = np.bincount(
        np.asarray(target).astype(np.int64), minlength=C
    ).astype(np.float64)
    return (np.abs(parts - counts[None, :]).sum(axis=1) / (B * C)).astype(
        np.float32
    )


# revision 28
# speedup vs baseline: 1.3740x; 1.1296x over previous
"""MDCA loss kernel for Trainium2, 8 NeuronCores, data-parallel over batch.

reference:
    counts[c]   = histogram(target) ; avg_count = counts/B
    avg_conf    = mean(logits, axis=1)            # [E, C]
    loss[e]     = mean_c |avg_conf[e,c] - avg_count[c]|

Device computes ONLY per-core column sums of logits (the 16.4 MB/core
stream, which binds at the ~430 GB/s per-core HBM-read ceiling); the
target histogram (8 KB) and final abs/mean run on host.  A bf16
DGE-cast variant was measured: the HBM-read side still caps ~420 GB/s
and SWDGE's Q7 descriptor emission adds a long trickle tail, so pure
HWDGE f32 wins.

Per core (batch shard of 1024 rows, partition p holds rows 8p..8p+7):
  - two HWDGE rings stream 16 KB contiguous lines; chunk sizes taper
    so the end-game is single-row f32r closers and four half-row
    closers at one matmul each.
  - DVE folds row pairs into [128,1000] bf16 tiles (1.2 us each,
    paced by arrivals); closers skip DVE.
  - ONE PSUM chain per 500-column half: selector matmuls ([128,4]
    with ones in column e) fold rows+partitions:
    psum[e,c] = sum over shard of logits[e,:,c]
  - tail: bank0 closes before bank1; copy0 on DVE + store on sync
    overlap bank1's close, whose copy runs on ACT + scalar.
  - host: bincount(target); |sum_conf - counts|.mean / (B*C) -> loss[4]
"""

import os
import sys

for _p in ("/opt/trn_rl_repo", "/root/.axon_site/_ro/trn_rl_repo"):
    if os.path.isdir(_p) and _p not in sys.path:
        sys.path.insert(0, _p)

import numpy as np

import concourse.bass as bass
import concourse.bacc as bacc
import concourse.tile as tile
import concourse.mybir as mybir
from concourse.bass_utils import run_bass_kernel_spmd

E, B, C = 4, 8192, 1000
N_CORES = 8
BS = B // N_CORES          # 1024 batch rows per core
GP = 8                     # rows folded per partition (BS = 128 * GP)
CH = C // 2                # 500, C half per PSUM bank
F32 = mybir.dt.float32
F32R = mybir.dt.float32r
BF16 = mybir.dt.bfloat16

# (exit, row0, row1, col0, col1) in issue order per ring.
RING_C = [(0, 4, 8, 0, C), (1, 4, 8, 0, C), (2, 4, 8, 0, C),
          (3, 4, 6, 0, C), (3, 6, 7, 0, C), (3, 7, 8, 0, C),
          (3, 3, 4, 0, CH), (3, 3, 4, CH, C)]
RING_S = [(0, 0, 4, 0, C), (1, 0, 4, 0, C), (2, 0, 4, 0, C),
          (3, 0, 2, 0, C), (3, 2, 3, 0, CH), (3, 2, 3, CH, C)]
# compute order ~ arrival order; half-closers land last
ORDER = ["c0", "s0", "c1", "s1", "c2", "s2", "c3", "s3", "c4", "c5",
         "s4", "c6", "s5", "c7"]


def build_nc():
    nc = bacc.Bacc(
        "TRN2",
        target_bir_lowering=False,
        debug=False,
        enable_asserts=False,
        num_devices=N_CORES,
    )

    logits = nc.dram_tensor("logits", [E, BS, C], F32, kind="ExternalInput")
    part_out = nc.dram_tensor("part", [E, C], F32, kind="ExternalOutput")

    # per-exit view: partition p holds rows 8p..8p+7
    src = [logits[e].rearrange("(p i) c -> p i c", i=GP) for e in range(E)]

    with tile.TileContext(nc) as tc:
        with (
            tc.tile_pool(name="const", bufs=1) as const,
            tc.tile_pool(name="ld4", bufs=6) as ld4,
            tc.tile_pool(name="ld2", bufs=3) as ld2,
            tc.tile_pool(name="ldz", bufs=4) as ldz,
            tc.tile_pool(name="fold", bufs=8) as foldp,
            tc.tile_pool(name="work", bufs=2) as work,
            tc.tile_pool(name="psum", bufs=1, space=bass.MemorySpace.PSUM) as psum,
        ):
            # ---- phase 1: every load DMA first so both rings fill and
            # stream back-to-back
            def ld_dma(engine, key, e, r0, r1, c0, c1):
                rows, cols = r1 - r0, c1 - c0
                if rows == 1:           # closer: f32r bitcast, no fold
                    pool, dt = ldz, F32R
                else:
                    pool, dt = (ld4 if rows == 4 else ld2), F32
                t = pool.tile([128, rows * cols], dt,
                              tag=f"ld{rows}_{cols}", name=f"ld_{key}")
                in_ = src[e][:, r0:r1, c0:c1]
                if dt is F32R:
                    in_ = in_.bitcast(F32R)
                engine.dma_start(
                    out=t.rearrange("p (i c) -> p i c", i=rows), in_=in_
                )
                return t

            tiles = {}
            for k, spec in enumerate(RING_C):
                tiles[f"c{k}"] = (ld_dma(nc.scalar, f"c{k}", *spec), spec)
            for k, spec in enumerate(RING_S):
                tiles[f"s{k}"] = (ld_dma(nc.sync, f"s{k}", *spec), spec)

            # ---- phase 2: selector weights (bf16 for folded tiles,
            # f32r for closers)
            sels_b = const.tile([128, 4 * E], BF16, tag="sels_b")
            nc.vector.memset(sels_b[:], 0.0)
            for e in range(E):
                nc.vector.memset(sels_b[:, 4 * e + e : 4 * e + e + 1], 1.0)
            sels_f = const.tile([128, 4 * E], F32, tag="sels_f")
            nc.vector.tensor_copy(sels_f[:], sels_b[:])
            # real F32R tile via copy — walrus's checkMatmultFP32r
            # rejects a bitcast AP as matmul weights
            sels_r_t = const.tile([128, 4 * E], F32R, tag="sels_r")
            nc.vector.tensor_copy(sels_r_t[:], sels_f[:])
            sels_r = sels_r_t[:]

            pbank = [
                psum.tile([E, CH], F32, tag=f"pc{h}", name=f"pc{h}")
                for h in range(2)
            ]

            # ---- phase 3: folds + matmuls in expected arrival order.
            # Precount per-bank matmuls so the last one sets stop=True.
            tot = [0, 0]
            for _, r0, r1, c0, c1 in RING_S + RING_C:
                rows = r1 - r0
                if c1 - c0 == CH:
                    tot[c0 // CH] += 1
                else:
                    tot[0] += rows // 2 + rows % 2
                    tot[1] += rows // 2 + rows % 2
            n_mm = [0, 0]

            def mm1(h, w, data):
                nc.tensor.matmul(
                    pbank[h][:], w, data,
                    start=(n_mm[h] == 0),
                    stop=(n_mm[h] == tot[h] - 1),
                )
                n_mm[h] += 1

            for key in ORDER:
                t, (e, r0, r1, c0, c1) = tiles[key]
                rows = r1 - r0
                wb = sels_b[:, 4 * e : 4 * e + 4]
                wr = sels_r[:, 4 * e : 4 * e + 4]
                if rows == 1 and c1 - c0 == CH:     # half-row closer
                    mm1(c0 // CH, wr, t[:, 0:CH])
                elif rows == 1:                     # full-row closer
                    for h in range(2):
                        mm1(h, wr, t[:, h * CH : (h + 1) * CH])
                else:
                    for g in range(rows // 2):
                        f = foldp.tile([128, C], BF16, tag="fold",
                                       name=f"f_{key}g{g}")
                        nc.vector.tensor_add(
                            f[:], t[:, 2 * g * C : (2 * g + 1) * C],
                            t[:, (2 * g + 1) * C : (2 * g + 2) * C],
                        )
                        for h in range(2):
                            mm1(h, wb, f[:, h * CH : (h + 1) * CH])

            # ---- phase 4: PSUM->SBUF on two engines + parallel stores
            sb0 = work.tile([E, CH], F32, tag="sb0")
            nc.vector.tensor_copy(sb0[:], pbank[0][:])
            nc.sync.dma_start(out=part_out[:, 0:CH], in_=sb0[:])
            # copy1 on ACT (parallel with DVE's copy0); BOTH stores issue
            # from sync — the scalar engine's store issue measured
            # 1.2-1.4 us vs sync's 0.78 us, and sync is idle by then
            sb1 = work.tile([E, CH], F32, tag="sb1")
            nc.scalar.copy(sb1[:], pbank[1][:])
            nc.sync.dma_start(out=part_out[:, CH:C], in_=sb1[:])

    nc.compile()
    return nc


_NC_CACHE = {}


def _get_nc():
    if "nc" not in _NC_CACHE:
        _NC_CACHE["nc"] = build_nc()
    return _NC_CACHE["nc"]


def make_in_maps(logits: np.ndarray, target: np.ndarray):
    logits = np.ascontiguousarray(logits, dtype=np.float32)
    in_maps = []
    for c in range(N_CORES):
        lg = logits[:, c * BS : (c + 1) * BS, :]
        in_maps.append({"logits": np.ascontiguousarray(lg)})
    return in_maps


def kernel(logits: np.ndarray, target: np.ndarray) -> np.ndarray:
    nc = _get_nc()
    in_maps = make_in_maps(logits, target)
    res = run_bass_kernel_spmd(nc, in_maps, core_ids=list(range(N_CORES)))
    parts = sum(np.asarray(r["part"], dtype=np.float64) for r in res.results)
    counts = np.bincount(
        np.asarray(target).astype(np.int64), minlength=C
    ).astype(np.float64)
    return (np.abs(parts - counts[None, :]).sum(axis=1) / (B * C)).astype(
        np.float32
    )
